# revision 4
# baseline (speedup 1.0000x reference)
"""Single-head attention (B=4, N=2048, D=1024), scores scaled by 10.

Sharding: 8 cores = (batch, query-half). Core 2b+h owns queries
[1024h:1024(h+1)] of batch b. K/V projections are computed for the OWN
half only and exchanged with the pair core (2b ^ 1) via an on-chip
AllGather, halving the projection FLOPs vs recomputing the full
sequence per core. Key order is global [h0|h1] (rank order) on every
core, so the SPMD program is identical across cores.

Numerics: everything runs single-pass fp16 (inputs rounded to fp16,
fp32 PSUM accumulation). Measured end-to-end rel err ~6e-3 against the
fp32 reference (gate 2e-2): the x10 score scale amplifies fp16 rounding
of Q/K into the softmax exponent, but the softmax is near one-hot
(score std ~107 after scaling) so only near-tie rows are affected.

Schedule: all weights are prefetched to SBUF before the collectives
start so the Q/K/V weight streams never contend with collective DMA
traffic. Phase order K proj -> AllGather K -> V proj -> AllGather V ->
Q proj puts both exchanges behind compute. The attention loop is
software-pipelined (QK of chunk c runs between softmax and PV of chunk
c-1) and the softmax cross-partition max/sum run on the idle GpSimd
engine (partition_all_reduce), so the tensor engine alternates QK/PV
blocks with no broadcast or reduction matmuls in between.
"""

import numpy as np

B, SEQ, D = 4, 2048, 1024
NQ = 1024          # queries per core (= keys computed per core)
QCH = 256          # attention q-chunk
NCH = NQ // QCH
NCORES = 8
DT = D // 128      # 8 d-tiles
ET = D // 128      # 8 e-tiles
KT = SEQ // 128    # 16 k-tiles
HKT = KT // 2      # 8 own-half k-tiles

_BUILT = {}


def _build():
    if "nc" in _BUILT:
        return _BUILT["nc"]
    from contextlib import ExitStack

    import concourse.bass as bass  # noqa: F401
    import concourse.mybir as mybir
    import concourse.tile as tile
    from concourse import bacc
    from concourse.bass_isa import ReduceOp

    dt = mybir.dt
    F32, F16 = dt.float32, dt.float16
    AL = mybir.AluOpType
    EXP = mybir.ActivationFunctionType.Exp
    GROUPS = [[2 * i, 2 * i + 1] for i in range(NCORES // 2)]

    nc = bacc.Bacc("TRN2", target_bir_lowering=False, debug=False)

    # x^T own half, packed [p, t, n]; weights packed so each e-tile DMA is
    # one contiguous 2KB line per partition
    xp_d = nc.dram_tensor("xp", [128, DT * NQ], F16, kind="ExternalInput")
    wq_d = nc.dram_tensor("wq", [128, ET * DT * 128], F16, kind="ExternalInput")
    wk_d = nc.dram_tensor("wk", [128, ET * DT * 128], F16, kind="ExternalInput")
    wv_d = nc.dram_tensor("wv", [128, DT * D], F16, kind="ExternalInput")
    ot_d = nc.dram_tensor("ot", [128, DT * NQ], F16, kind="ExternalOutput")

    xp_r = xp_d.ap().rearrange("p (t n) -> p t n", t=DT)
    wq_r = wq_d.ap().rearrange("p (e tc) -> p e tc", e=ET)
    wk_r = wk_d.ap().rearrange("p (e tc) -> p e tc", e=ET)
    wv_r = wv_d.ap().rearrange("p (t e) -> p t e", t=DT)
    ot_r = ot_d.ap().rearrange("p (t q) -> p t q", t=DT)

    with tile.TileContext(nc) as tc, ExitStack() as ctx:
        qk_pool = ctx.enter_context(tc.tile_pool(name="qk", bufs=1))
        qt = qk_pool.tile([128, ET, NQ], F16, tag="qt")
        ktt = qk_pool.tile([128, ET, SEQ], F16, tag="ktt")
        vf = qk_pool.tile([128, KT, D], F16, tag="vf")

        const_pool = ctx.enter_context(tc.tile_pool(name="const", bufs=1))

        dram = ctx.enter_context(tc.tile_pool(name="dram", bufs=1, space="DRAM"))
        k_in = dram.tile([D, NQ], F16, tag="k_in")
        k_out = dram.tile([2 * D, NQ], F16, tag="k_out")
        v_in = dram.tile([NQ, D], F16, tag="v_in")
        v_out = dram.tile([SEQ, D], F16, tag="v_out")
        warm_in = dram.tile([16, 16], F16, tag="warm_in")
        warm_out = dram.tile([32, 16], F16, tag="warm_out")

        # tiny warmup collective at t=0: pays the ncfw channel-setup latency
        # before the real exchanges need it
        warm_sb = const_pool.tile([16, 16], F16, tag="warm_sb")
        nc.vector.memset(warm_sb[:], 0.0)
        nc.sync.dma_start(warm_in[:], warm_sb[:])
        nc.gpsimd.collective_compute(
            "AllGather",
            AL.bypass,
            replica_groups=GROUPS,
            ins=[warm_in[:]],
            outs=[warm_out[:]],
        )

        # ---------------- Projections (all single-pass fp16) --------------
        with (
            tc.tile_pool(name="xspan", bufs=1) as xspan,
            tc.tile_pool(name="wall", bufs=1) as wall,
            tc.tile_pool(name="kev", bufs=4) as kevpool,
            tc.tile_pool(name="psA", bufs=4, space="PSUM") as psA,
        ):
            x_t = xspan.tile([128, DT, NQ], F16, tag="x")
            wkF = wall.tile([128, ET, DT * 128], F16, tag="wkF")
            wvF = wall.tile([128, DT, D], F16, tag="wvF")
            wqF = wall.tile([128, ET, DT * 128], F16, tag="wqF")
            # DMA order = need order: first K weight tile, x, rest of K
            # weights, V weights, Q weights. Everything is resident before
            # the first collective starts using the rings.
            nc.sync.dma_start(wkF[:, 0, :], wk_r[:, 0, :])
            for dti in range(DT):
                nc.sync.dma_start(x_t[:, dti, :], xp_r[:, dti, :])
            for et in range(1, ET):
                nc.sync.dma_start(wkF[:, et, :], wk_r[:, et, :])
            for ec in range(2):
                nc.sync.dma_start(
                    wvF[:, :, 512 * ec : 512 * (ec + 1)],
                    wv_r[:, :, 512 * ec : 512 * (ec + 1)],
                )
            for et in range(ET):
                nc.sync.dma_start(wqF[:, et, :], wq_r[:, et, :])

            # ---- Phase K: own-half K^T projection -----------------------
            for et in range(ET):
                e0 = 128 * et
                ps0 = psA.tile([128, 512], F32, tag="psA")
                ps1 = psA.tile([128, 512], F32, tag="psA")
                ps = (ps0, ps1)
                # dti outer so the first et paces with the incoming x stream
                for dti in range(DT):
                    for chn in range(2):
                        nc.tensor.matmul(
                            ps[chn][:],
                            wkF[:, et, 128 * dti : 128 * (dti + 1)],
                            x_t[:, dti, 512 * chn : 512 * (chn + 1)],
                            start=(dti == 0),
                            stop=(dti == DT - 1),
                        )
                for chn in range(2):
                    n0 = 512 * chn
                    kev = kevpool.tile([128, 512], F16, tag="kev")
                    nc.vector.tensor_copy(kev[:], ps[chn][:])
                    nc.sync.dma_start(k_in[e0 : e0 + 128, n0 : n0 + 512], kev[:])

            # pair AllGather of K halves
            nc.gpsimd.collective_compute(
                "AllGather",
                AL.bypass,
                replica_groups=GROUPS,
                ins=[k_in[:]],
                outs=[k_out[:]],
            )
            # readbacks dispatch as soon as the exchange lands
            k_out_r = k_out[:].rearrange("(h t p) n -> h p t n", p=128, t=ET)
            for h in range(2):
                nc.gpsimd.dma_start(
                    ktt[:, :, NQ * h : NQ * (h + 1)], k_out_r[h, :, :, :]
                )

            # ---- Phase V: own-half V projection --------------------------
            for ec in range(2):
                e0 = 512 * ec
                for kt in range(HKT):
                    k0 = 128 * kt
                    ps = psA.tile([128, 512], F32, tag="psA")
                    for dti in range(DT):
                        nc.tensor.matmul(
                            ps[:],
                            x_t[:, dti, k0 : k0 + 128],
                            wvF[:, dti, e0 : e0 + 512],
                            start=(dti == 0),
                            stop=(dti == DT - 1),
                        )
                    vev = kevpool.tile([128, 512], F16, tag="kev")
                    nc.vector.tensor_copy(vev[:], ps[:])
                    nc.sync.dma_start(v_in[k0 : k0 + 128, e0 : e0 + 512], vev[:])

            nc.gpsimd.collective_compute(
                "AllGather",
                AL.bypass,
                replica_groups=GROUPS,
                ins=[v_in[:]],
                outs=[v_out[:]],
            )
            v_out_r = v_out[:].rearrange("(h t p) e -> h p t e", p=128, t=HKT)
            for h in range(2):
                nc.gpsimd.dma_start(
                    vf[:, HKT * h : HKT * (h + 1), :], v_out_r[h, :, :, :]
                )

            # ---- Phase Q: own-half Q^T projection ------------------------
            for et in range(ET):
                for chn in range(2):
                    n0 = 512 * chn
                    ps = psA.tile([128, 512], F32, tag="psA")
                    for dti in range(DT):
                        nc.tensor.matmul(
                            ps[:],
                            wqF[:, et, 128 * dti : 128 * (dti + 1)],
                            x_t[:, dti, n0 : n0 + 512],
                            start=(dti == 0),
                            stop=(dti == DT - 1),
                        )
                    nc.vector.tensor_copy(qt[:, et, n0 : n0 + 512], ps[:])

        # ---------------- Attention, q-chunked, software-pipelined --------
        with (
            tc.tile_pool(name="stp", bufs=2) as stpool,
            tc.tile_pool(name="pp", bufs=2) as ppool,
            tc.tile_pool(name="tree", bufs=2) as treepool,
            tc.tile_pool(name="aux", bufs=2) as auxpool,
            tc.tile_pool(name="osb", bufs=3) as outpool,
            tc.tile_pool(name="psS", bufs=6, space="PSUM") as psS,
            tc.tile_pool(name="psO", bufs=2, space="PSUM") as psO,
        ):
            sts = [None] * NCH
            pts = [None] * NCH

            def qk_block(c):
                q0 = QCH * c
                st = stpool.tile([128, KT, QCH], F32, tag="st", name="st")
                sts[c] = st
                for kt in range(KT):
                    k0 = 128 * kt
                    ps = psS.tile([128, QCH], F32, tag="psS", name="ps")
                    for et in range(ET):
                        nc.tensor.matmul(
                            ps[:],
                            ktt[:, et, k0 : k0 + 128],
                            qt[:, et, q0 : q0 + QCH],
                            start=(et == 0),
                            stop=(et == ET - 1),
                        )
                    nc.vector.tensor_copy(st[:, kt, :], ps[:])

            def softmax_head(c):
                # rowwise max over kt on DVE, then cross-partition max +
                # broadcast on GpSimd, then exp(10*(s - max)) on Act
                st = sts[c]
                t8 = treepool.tile([128, 8, QCH], F32, tag="t8", name="t8")
                for j in range(8):
                    nc.vector.tensor_max(
                        t8[:, j, :], st[:, 2 * j, :], st[:, 2 * j + 1, :]
                    )
                for j in range(4):
                    nc.vector.tensor_max(
                        t8[:, j, :], t8[:, 2 * j, :], t8[:, 2 * j + 1, :]
                    )
                nc.vector.tensor_max(t8[:, 0, :], t8[:, 0, :], t8[:, 1, :])
                nc.vector.tensor_max(t8[:, 2, :], t8[:, 2, :], t8[:, 3, :])
                nc.vector.tensor_max(t8[:, 0, :], t8[:, 0, :], t8[:, 2, :])
                maxb = auxpool.tile([128, QCH], F32, tag="maxb", name="maxb")
                nc.gpsimd.partition_all_reduce(
                    maxb[:], t8[:, 0, :], 128, ReduceOp.max
                )
                p_t = ppool.tile([128, KT, QCH], F16, tag="p", name="p_t")
                pts[c] = p_t
                for kt in range(KT):
                    nc.vector.tensor_sub(st[:, kt, :], st[:, kt, :], maxb[:])
                    nc.scalar.activation(
                        p_t[:, kt, :], st[:, kt, :], EXP, scale=10.0
                    )

            def sums_pv(c):
                q0 = QCH * c
                p_t = pts[c]
                # sum over keys: kt-tree on DVE (fp16), partitions on GpSimd
                s8 = treepool.tile([128, 8, QCH], F16, tag="s8", name="s8")
                for j in range(8):
                    nc.vector.tensor_add(
                        s8[:, j, :], p_t[:, 2 * j, :], p_t[:, 2 * j + 1, :]
                    )
                for j in range(4):
                    nc.vector.tensor_add(
                        s8[:, j, :], s8[:, 2 * j, :], s8[:, 2 * j + 1, :]
                    )
                nc.vector.tensor_add(s8[:, 0, :], s8[:, 0, :], s8[:, 1, :])
                nc.vector.tensor_add(s8[:, 2, :], s8[:, 2, :], s8[:, 3, :])
                nc.vector.tensor_add(s8[:, 0, :], s8[:, 0, :], s8[:, 2, :])
                sumb = auxpool.tile([128, QCH], F32, tag="sumb", name="sumb")
                nc.gpsimd.partition_all_reduce(
                    sumb[:], s8[:, 0, :], 128, ReduceOp.add
                )
                recb = auxpool.tile([128, QCH], F32, tag="recb", name="recb")
                nc.vector.reciprocal(recb[:], sumb[:])
                # O^T[d, q] = V^T P, scaled by 1/sum
                for dti in range(DT):
                    d0 = 128 * dti
                    ops = psO.tile([128, QCH], F32, tag="psO", name="ops")
                    for kt in range(KT):
                        nc.tensor.matmul(
                            ops[:],
                            vf[:, kt, d0 : d0 + 128],
                            p_t[:, kt, :],
                            start=(kt == 0),
                            stop=(kt == KT - 1),
                        )
                    osb = outpool.tile([128, QCH], F16, tag="osb", name="osb")
                    nc.vector.scalar_tensor_tensor(
                        osb[:], ops[:], 1.0, recb[:], op0=AL.mult, op1=AL.mult
                    )
                    nc.sync.dma_start(ot_r[:, dti, q0 : q0 + QCH], osb[:])

            # pipeline: QK(c) fills the tensor engine while softmax(c-1)
            # runs on DVE/GpSimd/Act; the last chunk's softmax is hoisted
            # before PV(c-2) so its exp is ready when the PE drains
            qk_block(0)
            for c in range(1, NCH):
                softmax_head(c - 1)
                qk_block(c)
                if c == NCH - 1:
                    softmax_head(c)
                sums_pv(c - 1)
            sums_pv(NCH - 1)

    nc.compile()
    _BUILT["nc"] = nc
    return nc


def _prep_inputs(x, q_w, k_w, v_w):
    f16 = np.float16

    def pack_w_lhsT(w):
        # w is [out=e, in=d]; pack [p, eb, t, c] = w[eb*128+c, t*128+p]
        a = w.T.astype(f16).reshape(DT, 128, ET, 128)
        return np.ascontiguousarray(a.transpose(1, 2, 0, 3)).reshape(
            128, ET * DT * 128
        )

    def pack_w_rhs(w):
        # pack [p, t, e] = w.T[t*128+p, e]
        a = w.T.astype(f16).reshape(DT, 128, D)
        return np.ascontiguousarray(a.transpose(1, 0, 2)).reshape(128, DT * D)

    wq = pack_w_lhsT(q_w)
    wk = pack_w_lhsT(k_w)
    wv = pack_w_rhs(v_w)

    in_maps = []
    for core in range(NCORES):
        b, h = divmod(core, 2)
        xt = np.asarray(x[b, NQ * h : NQ * (h + 1)]).T.astype(f16)  # [d, n]
        xp = np.ascontiguousarray(
            xt.reshape(DT, 128, NQ).transpose(1, 0, 2)
        ).reshape(128, DT * NQ)
        in_maps.append({"xp": xp, "wq": wq, "wk": wk, "wv": wv})
    return in_maps


def run(x, q_w, k_w, v_w, trace=False):
    from concourse.bass_utils import run_bass_kernel_spmd

    nc = _build()
    in_maps = _prep_inputs(x, q_w, k_w, v_w)
    res = run_bass_kernel_spmd(nc, in_maps, list(range(NCORES)), trace=trace)
    out = np.empty((B, SEQ, D), np.float32)
    for core in range(NCORES):
        b, h = divmod(core, 2)
        ot = res.results[core]["ot"].astype(np.float32).reshape(128, DT, NQ)
        out[b, NQ * h : NQ * (h + 1)] = (
            ot.transpose(1, 0, 2).reshape(D, NQ).T
        )
    return out, res


def kernel(x, q_w, k_w, v_w):
    x = np.asarray(x, np.float32)
    q_w = np.asarray(q_w, np.float32)
    k_w = np.asarray(k_w, np.float32)
    v_w = np.asarray(v_w, np.float32)
    out, _ = run(x, q_w, k_w, v_w, trace=False)
    return out


if __name__ == "__main__":
    rng = np.random.default_rng(0)
    x = rng.standard_normal((B, SEQ, D), np.float32)
    s = 1.0 / np.sqrt(D)
    q_w = rng.uniform(-s, s, (D, D)).astype(np.float32)
    k_w = rng.uniform(-s, s, (D, D)).astype(np.float32)
    v_w = rng.uniform(-s, s, (D, D)).astype(np.float32)
    out = kernel(x, q_w, k_w, v_w)
    print(out.shape, out.dtype)
